# revision 31
# baseline (speedup 1.0000x reference)
"""Multi-head attention (B=4, T=2048, dim=2048, H=16, RoPE) on 8 TRN2 NeuronCores.

Tensor-parallel over heads: core c owns heads {2c, 2c+1} (projection dim
slice [256c, 256c+256)).  Each core computes q/k/v projections for its
heads, RoPE, full softmax attention for its 8 (batch, head) pairs, and a
partial output projection against its 256-row slice of wo; the host sums
the 8 partial outputs (bf16) and adds wo_b.

All matmuls run in bf16 with f32 PSUM accumulation; softmax runs exp in
f32->bf16 on the scalar engine with denominators accumulated via an extra
ones-column on V through the PV matmul.

Engine assignment: PE does all matmuls/transposes; Activation does exp
(the critical softmax chain) plus half the out-proj PSUM drains; DVE does
RoPE, reciprocals, PV scale, aoT copies, and the other out-proj drains;
gpsimd issues the weight/x DMAs and half the output DMAs (it cannot touch
PSUM on TRN2).  The attention loop is span-outer and PV for each
half-span is deferred into the next half-span's S/exp block (chains
interleaved at odd kt2) so the PE never stalls on the exp pipeline;
deferred out-proj groups of the previous batch fill the remaining slots,
and the final batch's own out-proj drips per token-tile at the tail.
"""

import json
import sys

sys.path.insert(0, "/opt/trn_rl_repo")

import ml_dtypes
import numpy as np

BF16 = ml_dtypes.bfloat16

# Problem shape (hardcoded per contract).
B, T, D = 4, 2048, 2048
H = 16
N_CORES = 8
HL = H // N_CORES  # heads per core = 2
DH = D // H  # head dim = 128
DOUT = HL * DH  # per-core projection width = 256
BT = B * T  # 8192 tokens
P = 128
NK = D // P  # 16 feature chunks
SPAN = 512
NSPAN = T // SPAN  # 4 token spans per batch
NTT = T // P  # 16 token tiles per batch


# ---------------------------------------------------------------------------
# BIR legalization: the walrus build in this container rejects instructions
# carrying more than one sync wait. Engines execute their stream in order, so
# hoisting excess waits into standalone EventSemaphore instructions directly
# before the instruction (same engine) is semantically equivalent; Tile's
# dependency graph is acyclic so this cannot deadlock.
# ---------------------------------------------------------------------------


def _legalize_waits(bir_json: bytes, max_inline: int = 1, es_capacity: int = 2):
    bir = json.loads(bir_json)
    for f in bir.get("functions", []):
        for bb in f.get("blocks", []):
            out = []
            for inst in bb.get("instructions", []):
                si = inst.get("sync_info")
                waits = (si or {}).get("on_wait") or []
                cap = (
                    es_capacity
                    if inst.get("opcode") == "EventSemaphore"
                    else max_inline
                )
                if len(waits) > cap:
                    keep, excess = waits[:cap], waits[cap:]
                    for ci in range(0, len(excess), es_capacity):
                        out.append(
                            {
                                "debug": inst.get("debug", 0),
                                "engine": inst["engine"],
                                "ins": [],
                                "name": f"{inst['name']}_xw{ci}",
                                "opcode": "EventSemaphore",
                                "outs": [],
                                "sync_info": {
                                    "on_update": [],
                                    "on_wait": excess[ci : ci + es_capacity],
                                },
                            }
                        )
                    si["on_wait"] = keep
                out.append(inst)
            bb["instructions"] = out
    return json.dumps(bir).encode()


_patched = False


def _install_compile_patch():
    global _patched
    if _patched:
        return
    _patched = True
    from concourse import bass2jax, bass_utils

    orig = bass_utils.compile_bir_kernel

    def patched_compile(bir_json, tmpdir, neff_name="file.neff"):
        return orig(_legalize_waits(bir_json), tmpdir, neff_name)

    bass2jax.compile_bir_kernel = patched_compile


# ---------------------------------------------------------------------------
# Kernel builder (one SPMD graph; per-core behavior differs only via inputs)
# ---------------------------------------------------------------------------


def _build_nc():
    import concourse.bass as bass
    import concourse.tile as tile
    from concourse import mybir
    from concourse.masks import make_identity

    f32 = mybir.dt.float32
    bf16 = mybir.dt.bfloat16

    nc = bass.Bass()
    xT = nc.declare_dram_parameter("xT", [D, BT], bf16, isOutput=False)
    wqT = nc.declare_dram_parameter("wqT", [D + 1, DOUT], bf16, isOutput=False)
    wkT = nc.declare_dram_parameter("wkT", [D + 1, DOUT], bf16, isOutput=False)
    wvT = nc.declare_dram_parameter("wvT", [D + 1, DOUT], bf16, isOutput=False)
    woT = nc.declare_dram_parameter("woT", [DOUT, D], bf16, isOutput=False)
    cosT = nc.declare_dram_parameter("cosT", [DH // 2, T], bf16, isOutput=False)
    sinT = nc.declare_dram_parameter("sinT", [DH // 2, T], bf16, isOutput=False)
    outp = nc.declare_dram_parameter("out", [BT, D], bf16, isOutput=True)

    HDH = DH + 1  # head slot width in v_ones (128 v cols + ones col)
    hh = DH // 2
    Copy = mybir.ActivationFunctionType.Copy

    with tile.TileContext(nc) as tc:
        with (
            tc.tile_pool(name="wpool", bufs=1) as wpool,
            tc.tile_pool(name="xpool", bufs=2) as xpool,
            tc.tile_pool(name="qkv", bufs=1) as qkv,
            tc.tile_pool(name="qkT", bufs=1) as qkT,
            tc.tile_pool(name="aop", bufs=2) as aop,
            tc.tile_pool(name="epool", bufs=2) as epool,
            tc.tile_pool(name="misc", bufs=2) as misc,
            tc.tile_pool(name="recp", bufs=4) as recp,
            tc.tile_pool(name="obuf", bufs=6) as obuf,
            tc.tile_pool(name="psS", bufs=2, space="PSUM") as psS,
            tc.tile_pool(name="ps512", bufs=2, space="PSUM") as ps512,
            tc.tile_pool(name="pspv", bufs=2, space="PSUM") as pspv,
        ):
            # ---- persistent: weights, tables, identity, bias columns ----
            ident = wpool.tile([P, P], bf16, tag="ident")
            make_identity(nc, ident)

            def load_wT(name, dram):
                # one DMA for all 16 k-chunks: [2048, DOUT] -> [128, 16, DOUT]
                wsb = wpool.tile([P, NK, DOUT], bf16, tag=name)
                wsrc = dram[:D, :].rearrange("(ko p) d -> p ko d", p=P)
                for c4 in range(4):
                    nc.gpsimd.dma_start(
                        out=wsb[:, c4 * 4 : (c4 + 1) * 4, :],
                        in_=wsrc[:, c4 * 4 : (c4 + 1) * 4, :],
                    )
                # bias as per-partition half-columns (base partition 0), per head
                bcols = []
                for m in range(HL):
                    halves = []
                    for h2 in range(2):
                        bc = wpool.tile([hh, 1], bf16, tag=f"{name}bc{m}_{h2}")
                        nc.gpsimd.dma_start(
                            out=bc,
                            in_=dram[D : D + 1, m * P + h2 * hh : m * P + (h2 + 1) * hh],
                        )
                        halves.append(bc)
                    bcols.append(halves)
                return wsb, bcols

            # k path loads first: attention's S matmuls need the FULL kT, so k
            # projections run before q each batch and their weights must land
            # first at startup. cos/sin go on the idle sync queue so RoPE
            # unblocks early without delaying the wk/x chunks.
            wk_t, wk_bc = load_wT("wk", wkT)
            cos_sb = wpool.tile([hh, T], bf16, tag="cos")
            sin_sb = wpool.tile([hh, T], bf16, tag="sin")
            deferred = {}

            def load_rest():
                wq_t, wq_bc = load_wT("wq", wqT)
                wv_t, _ = load_wT("wv", wvT)
                # v bias broadcast tile [P, DOUT] from the wvT bias row
                vb_bc = wpool.tile([P, DOUT], bf16, tag="vb_bc")
                wvT_brow = wvT[D : D + 1, :]
                nc.gpsimd.dma_start(
                    out=vb_bc,
                    in_=bass.AP(
                        tensor=wvT_brow.tensor,
                        offset=wvT_brow.offset,
                        ap=[[0, P], wvT_brow.ap[-1]],
                    ),
                )
                deferred.update(
                    wq_t=wq_t, wq_bc=wq_bc, wv_t=wv_t, vb_bc=vb_bc,
                )

            wo_t = []
            op_alt = [0]

            def emit_outproj_group(bb, tt, ds):
                t0 = bb * T + tt * P
                ps = ps512.tile([P, SPAN], f32, tag="p512", name="ps_op")
                for m in range(HL):
                    nc.tensor.matmul(
                        ps,
                        aoT_of[bb % 2][m][:, tt * P : (tt + 1) * P],
                        wo_t[m][:, ds * SPAN : (ds + 1) * SPAN],
                        start=(m == 0),
                        stop=(m == HL - 1),
                    )
                ob = obuf.tile([P, SPAN], bf16, tag="ob")
                op_alt[0] ^= 1
                if op_alt[0]:
                    nc.vector.tensor_copy(out=ob, in_=ps)
                    dma_eng = nc.sync
                else:
                    nc.scalar.copy(out=ob, in_=ps)
                    dma_eng = nc.gpsimd
                dma_eng.dma_start(
                    out=outp[t0 : t0 + P, ds * SPAN : (ds + 1) * SPAN], in_=ob
                )

            aoT_of = {}
            add = mybir.AluOpType.add
            mult = mybir.AluOpType.mult
            for b in range(B):
                # ---- QKV projections + RoPE, processed in span pairs ----
                qT = [qkT.tile([P, T], bf16, tag=f"qT{m}", name=f"qT{m}") for m in range(HL)]
                kT = [qkT.tile([P, T], bf16, tag=f"kT{m}", name=f"kT{m}") for m in range(HL)]
                v_t = [qkv.tile([P, HL * HDH], bf16, tag=f"v{tt}", name=f"v{tt}") for tt in range(NTT)]
                ao_t = [qkv.tile([P, DOUT], bf16, tag=f"ao{tt}", name=f"ao{tt}") for tt in range(NTT)]
                aoT_of[b % 2] = [aop.tile([P, T], bf16, tag=f"aoT{m}", name=f"aoT{m}") for m in range(HL)]

                xks = []
                for s2 in range(NSPAN // 2):
                    t0 = b * T + 2 * s2 * SPAN
                    xk = xpool.tile([P, NK, 2 * SPAN], bf16, tag="x", name="x")
                    xsrc = xT[:, t0 : t0 + 2 * SPAN].rearrange(
                        "(ko p) t -> p ko t", p=P
                    )
                    for c4 in range(4):
                        # at cold start split the first span-pair across both
                        # queues so the first k-projection unblocks sooner
                        eng = (
                            nc.sync
                            if (b == 0 and s2 == 0 and c4 >= 2)
                            else nc.gpsimd
                        )
                        eng.dma_start(
                            out=xk[:, c4 * 4 : (c4 + 1) * 4, :],
                            in_=xsrc[:, c4 * 4 : (c4 + 1) * 4, :],
                        )
                    xks.append(xk)
                    if not deferred:
                        # cos/sin ride the sync queue behind the first x
                        # chunks; RoPE doesn't need them until ~8us in
                        nc.sync.dma_start(out=cos_sb, in_=cosT[:, :])
                        nc.sync.dma_start(out=sin_sb, in_=sinT[:, :])
                        load_rest()
                if b == 0:
                    wq_t = deferred["wq_t"]
                    wq_bc = deferred["wq_bc"]
                    wv_t = deferred["wv_t"]
                    vb_bc = deferred["vb_bc"]

                # k/q over both span pairs (k first: S needs the full kT),
                # [128, 1024] psum per (dst, m, span-pair)
                for dst, wsb, bcols in ((kT, wk_t, wk_bc), (qT, wq_t, wq_bc)):
                    for s2 in range(NSPAN // 2):
                        xk = xks[s2]
                        sl2 = slice(2 * s2 * SPAN, (2 * s2 + 2) * SPAN)
                        for m in range(HL):
                            ps = psS.tile([P, 2 * SPAN], f32, tag="pS", name="ps_qk")
                            for k in range(NK):
                                for half in range(2):
                                    nc.tensor.matmul(
                                        ps[:, half * SPAN : (half + 1) * SPAN],
                                        wsb[:, k, m * P : (m + 1) * P],
                                        xk[:, k, half * SPAN : (half + 1) * SPAN],
                                        start=(k == 0),
                                        stop=(k == NK - 1),
                                    )
                            # RoPE over 1024 cols with bias fold:
                            #   qb = psum + bias; out[0:64] = qb0*cos - qb1*sin
                            #   out[64:128] = qb1*cos + qb0*sin
                            cs = cos_sb[:, sl2]
                            sn = sin_sb[:, sl2]
                            p0 = ps[0:hh, :]
                            p1 = ps[hh : 2 * hh, :]
                            b0 = bcols[m][0]
                            b1 = bcols[m][1]
                            t1 = misc.tile([hh, 2 * SPAN], bf16, tag="r1", name="t1")
                            t2 = misc.tile([hh, 2 * SPAN], bf16, tag="r2", name="t2")
                            nc.vector.scalar_tensor_tensor(t1, p1, b1, sn, add, mult)
                            nc.vector.scalar_tensor_tensor(t2, p0, b0, cs, add, mult)
                            nc.vector.tensor_sub(dst[m][0:hh, sl2], t2, t1)
                            t3 = misc.tile([hh, 2 * SPAN], bf16, tag="r1", name="t3")
                            t4 = misc.tile([hh, 2 * SPAN], bf16, tag="r2", name="t4")
                            nc.vector.scalar_tensor_tensor(t3, p0, b0, sn, add, mult)
                            nc.vector.scalar_tensor_tensor(t4, p1, b1, cs, add, mult)
                            nc.vector.tensor_add(dst[m][hh : 2 * hh, sl2], t4, t3)

                # v: per 128-token tile
                for s2 in range(NSPAN // 2):
                    xk = xks[s2]
                    for half in range(2):
                        s = 2 * s2 + half
                        for tt in range(SPAN // P):
                            gt = s * (SPAN // P) + tt
                            sl_p = slice(
                                half * SPAN + tt * P, half * SPAN + (tt + 1) * P
                            )
                            ps = ps512.tile([P, SPAN], f32, tag="p512", name="ps_v")
                            psv = ps[:, :DOUT]
                            for k in range(NK):
                                nc.tensor.matmul(
                                    psv,
                                    xk[:, k, sl_p],
                                    wv_t[:, k, :],
                                    start=(k == 0),
                                    stop=(k == NK - 1),
                                )
                            vt = v_t[gt]
                            ones_ap = bass.AP(
                                tensor=vt.tensor,
                                offset=vt.offset + DH,
                                ap=[vt.ap[0], [HDH, HL]],
                            )
                            nc.vector.memset(ones_ap, 1.0)
                            for m in range(HL):
                                nc.vector.tensor_add(
                                    vt[:, m * HDH : m * HDH + DH],
                                    psv[:, m * DH : (m + 1) * DH],
                                    vb_bc[:, m * DH : (m + 1) * DH],
                                )

                if b == 0:
                    # wo needed only from the first out-proj (during attn of b=1);
                    # late code position keeps it off the critical startup path.
                    for m in range(HL):
                        t = wpool.tile([P, D], bf16, tag=f"wo{m}")
                        nc.gpsimd.dma_start(out=t, in_=woT[m * P : (m + 1) * P, :])
                        wo_t.append(t)

                # ---- attention (b), span-outer, with out-proj interleaved ----
                # op_work: deferred out-proj groups of batch b-1, plus (for the
                # final batch) batch b's own groups appended as spans complete.
                op_work = (
                    [(b - 1, tt, ds) for tt in range(NTT) for ds in range(D // SPAN)]
                    if b > 0
                    else []
                )
                op_i = [0]
                op_rate = 2 if b == B - 1 else 1

                def emit_ops(n):
                    while op_i[0] < len(op_work) and n > 0:
                        emit_outproj_group(*op_work[op_i[0]])
                        op_i[0] += 1
                        n -= 1

                def pv_chain(m, s, etiles, tt):
                    gt = s * (SPAN // P) + tt
                    sl_p = slice(tt * P, (tt + 1) * P)
                    po = pspv.tile([P, DH + 1], f32, tag="pv", name="po")
                    for kt in range(NTT):
                        nc.tensor.matmul(
                            po,
                            etiles[kt][:, sl_p],
                            v_t[kt][:, m * HDH : (m + 1) * HDH],
                            start=(kt == 0),
                            stop=(kt == NTT - 1),
                        )
                    rec = recp.tile([P, 1], f32, tag="rec")
                    nc.vector.reciprocal(rec, po[:, DH : DH + 1])
                    nc.vector.tensor_scalar_mul(
                        ao_t[gt][:, m * DH : (m + 1) * DH],
                        po[:, 0:DH],
                        rec,
                    )

                def finish_span(s):
                    # transpose span s attn_out: [t, d] -> [d, t] via the DMA
                    # xbar (frees PE and DVE); both heads' PV for span s have
                    # completed by the time this is called
                    for tt4 in range(SPAN // P):
                        tt = s * (SPAN // P) + tt4
                        for m in range(HL):
                            nc.sync.dma_start_transpose(
                                out=aoT_of[b % 2][m][:, tt * P : (tt + 1) * P],
                                in_=ao_t[tt][:, m * DH : (m + 1) * DH],
                            )
                    if b == B - 1:
                        # final batch: its own out-proj drips right behind each
                        # span so the tail overlaps instead of serializing
                        for tt4 in range(SPAN // P):
                            tt = s * (SPAN // P) + tt4
                            for ds in range(D // SPAN):
                                op_work.append((b, tt, ds))

                # PV for half-span (m, s) is deferred into the NEXT half-span's
                # S/exp block (chains interleaved at odd kt2) so its exp inputs
                # are long since drained and the PE never starves on the psS
                # ring while the scalar engine works through the exps.
                pending_pv = None
                for s in range(NSPAN):
                    sl_q = slice(s * SPAN, (s + 1) * SPAN)
                    for m in range(HL):
                        # S.T pairs: two k-tiles per [P, 1024] psum -> one exp op
                        etiles = []
                        for kt2 in range(NTT // 2):
                            ps = psS.tile([P, 2 * SPAN], f32, tag="pS", name="ps_s")
                            for half in range(2):
                                nc.tensor.matmul(
                                    ps[:, half * SPAN : (half + 1) * SPAN],
                                    kT[m][:, (2 * kt2 + half) * P : (2 * kt2 + half + 1) * P],
                                    qT[m][:, sl_q],
                                    start=True,
                                    stop=True,
                                )
                            e = epool.tile([P, 2 * SPAN], bf16, tag=f"e{kt2}")
                            nc.scalar.activation(
                                out=e, in_=ps, func=mybir.ActivationFunctionType.Exp
                            )
                            etiles.append(e[:, 0:SPAN])
                            etiles.append(e[:, SPAN : 2 * SPAN])
                            # keep PE fed while exp drains the S psum slots
                            emit_ops(op_rate)
                            if kt2 % 2 == 1 and pending_pv is not None:
                                pv_chain(*pending_pv, kt2 // 2)
                        if pending_pv is not None and pending_pv[0] == HL - 1:
                            finish_span(pending_pv[1])
                        pending_pv = (m, s, etiles)
                # batch tail: final half-span's chains, transposes, and (for
                # the last batch) out-proj groups interleaved per token-tile
                # so PE work overlaps the trailing DVE/DMA drains.
                s_l = NSPAN - 1
                for tt4 in range(SPAN // P):
                    pv_chain(*pending_pv, tt4)
                    tt = s_l * (SPAN // P) + tt4
                    for m in range(HL):
                        nc.sync.dma_start_transpose(
                            out=aoT_of[b % 2][m][:, tt * P : (tt + 1) * P],
                            in_=ao_t[tt][:, m * DH : (m + 1) * DH],
                        )
                    if b == B - 1:
                        for ds in range(D // SPAN):
                            op_work.append((b, tt, ds))
                        emit_ops(D // SPAN)
                emit_ops(len(op_work))
    return nc


_nc_cache = None


def _get_nc():
    global _nc_cache
    if _nc_cache is None:
        _nc_cache = _build_nc()
    return _nc_cache


# ---------------------------------------------------------------------------
# Host wrapper
# ---------------------------------------------------------------------------


def _prep_inputs(x, pos, wq_w, wq_b, wk_w, wk_b, wv_w, wv_b, wo_w, wo_b):
    x2 = np.asarray(x, np.float32).reshape(BT, D)
    xT = np.ascontiguousarray(x2.T).astype(BF16)

    pos1 = np.asarray(pos, np.float32).reshape(T)
    freq = (1.0 / 10000.0 ** (np.arange(0, DH, 2, np.float32) / DH)).astype(np.float32)
    ang = pos1[None, :] * freq[:, None]  # [64, T]
    cosT = np.cos(ang).astype(BF16)
    sinT = np.sin(ang).astype(BF16)

    scale = np.float32(1.0 / np.sqrt(DH))

    def wslice(w, bvec, c, s=None):
        w = np.asarray(w, np.float32)
        bvec = np.asarray(bvec, np.float32)
        ws = w[c * DOUT : (c + 1) * DOUT]  # [256, D]
        bs = bvec[c * DOUT : (c + 1) * DOUT]
        if s is not None:
            ws = ws * s
            bs = bs * s
        out = np.empty((D + 1, DOUT), BF16)
        out[:D] = ws.T.astype(BF16)
        out[D] = bs.astype(BF16)
        return out

    in_maps = []
    for c in range(N_CORES):
        woTc = (
            np.asarray(wo_w, np.float32)[:, c * DOUT : (c + 1) * DOUT]
            .T.astype(BF16)
            .copy()
        )
        in_maps.append(
            {
                "xT": xT,
                "wqT": wslice(wq_w, wq_b, c, scale),
                "wkT": wslice(wk_w, wk_b, c),
                "wvT": wslice(wv_w, wv_b, c),
                "woT": woTc,
                "cosT": cosT,
                "sinT": sinT,
            }
        )
    return in_maps


def _run(in_maps, trace=False):
    _install_compile_patch()
    from concourse.bass_utils import run_bass_kernel_spmd

    nc = _get_nc()
    return run_bass_kernel_spmd(
        nc, in_maps, core_ids=list(range(N_CORES)), trace=trace
    )


def kernel(**inputs):
    inputs = {k: np.asarray(v) for k, v in inputs.items()}
    in_maps = _prep_inputs(**inputs)
    r = _run(in_maps, trace=False)
    acc = np.zeros((BT, D), np.float32)
    for c in range(N_CORES):
        acc += r.results[c]["out"].astype(np.float32)
    acc += np.asarray(inputs["wo_b"], np.float32)
    return acc.reshape(B, T, D)


# revision 32
# speedup vs baseline: 1.1163x; 1.1163x over previous
"""Multi-head attention (B=4, T=2048, dim=2048, H=16, RoPE) on 8 TRN2 NeuronCores.

Tensor-parallel over heads: core c owns heads {2c, 2c+1} (projection dim
slice [256c, 256c+256)).  Each core computes q/k/v projections for its
heads, RoPE, full softmax attention for its 8 (batch, head) pairs, and a
partial output projection against its 256-row slice of wo; the host sums
the 8 partial outputs (bf16) and adds wo_b.

All matmuls run in bf16 with f32 PSUM accumulation; softmax runs exp in
f32->bf16 on the scalar engine with denominators accumulated via an extra
ones-column on V through the PV matmul.

Engine assignment: PE does all matmuls/transposes; Activation does exp
(the critical softmax chain) plus half the out-proj PSUM drains; DVE does
RoPE, reciprocals, PV scale, aoT copies, and the other out-proj drains;
gpsimd issues the weight/x DMAs and half the output DMAs (it cannot touch
PSUM on TRN2).  The attention loop is span-outer and PV for each
half-span is deferred into the next half-span's S/exp block (chains
interleaved at odd kt2) so the PE never stalls on the exp pipeline;
deferred out-proj groups of the previous batch fill the remaining slots,
and the final batch's own out-proj drips per token-tile at the tail.
"""

import json
import sys

sys.path.insert(0, "/opt/trn_rl_repo")

import ml_dtypes
import numpy as np

BF16 = ml_dtypes.bfloat16

# Problem shape (hardcoded per contract).
B, T, D = 4, 2048, 2048
H = 16
N_CORES = 8
HL = H // N_CORES  # heads per core = 2
DH = D // H  # head dim = 128
DOUT = HL * DH  # per-core projection width = 256
BT = B * T  # 8192 tokens
P = 128
NK = D // P  # 16 feature chunks
SPAN = 512
NSPAN = T // SPAN  # 4 token spans per batch
NTT = T // P  # 16 token tiles per batch


# ---------------------------------------------------------------------------
# BIR legalization: the walrus build in this container rejects instructions
# carrying more than one sync wait. Engines execute their stream in order, so
# hoisting excess waits into standalone EventSemaphore instructions directly
# before the instruction (same engine) is semantically equivalent; Tile's
# dependency graph is acyclic so this cannot deadlock.
# ---------------------------------------------------------------------------


def _legalize_waits(bir_json: bytes, max_inline: int = 1, es_capacity: int = 2):
    bir = json.loads(bir_json)
    for f in bir.get("functions", []):
        for bb in f.get("blocks", []):
            out = []
            for inst in bb.get("instructions", []):
                si = inst.get("sync_info")
                waits = (si or {}).get("on_wait") or []
                cap = (
                    es_capacity
                    if inst.get("opcode") == "EventSemaphore"
                    else max_inline
                )
                if len(waits) > cap:
                    keep, excess = waits[:cap], waits[cap:]
                    for ci in range(0, len(excess), es_capacity):
                        out.append(
                            {
                                "debug": inst.get("debug", 0),
                                "engine": inst["engine"],
                                "ins": [],
                                "name": f"{inst['name']}_xw{ci}",
                                "opcode": "EventSemaphore",
                                "outs": [],
                                "sync_info": {
                                    "on_update": [],
                                    "on_wait": excess[ci : ci + es_capacity],
                                },
                            }
                        )
                    si["on_wait"] = keep
                out.append(inst)
            bb["instructions"] = out
    return json.dumps(bir).encode()


_patched = False


def _install_compile_patch():
    global _patched
    if _patched:
        return
    _patched = True
    from concourse import bass2jax, bass_utils

    orig = bass_utils.compile_bir_kernel

    def patched_compile(bir_json, tmpdir, neff_name="file.neff"):
        return orig(_legalize_waits(bir_json), tmpdir, neff_name)

    bass2jax.compile_bir_kernel = patched_compile


# ---------------------------------------------------------------------------
# Kernel builder (one SPMD graph; per-core behavior differs only via inputs)
# ---------------------------------------------------------------------------


def _build_nc():
    import concourse.bass as bass
    import concourse.tile as tile
    from concourse import mybir
    from concourse.masks import make_identity

    f32 = mybir.dt.float32
    bf16 = mybir.dt.bfloat16

    nc = bass.Bass()
    xT = nc.declare_dram_parameter("xT", [D, BT], bf16, isOutput=False)
    wqT = nc.declare_dram_parameter("wqT", [D + 1, DOUT], bf16, isOutput=False)
    wkT = nc.declare_dram_parameter("wkT", [D + 1, DOUT], bf16, isOutput=False)
    wvT = nc.declare_dram_parameter("wvT", [D + 1, DOUT], bf16, isOutput=False)
    woT = nc.declare_dram_parameter("woT", [DOUT, D], bf16, isOutput=False)
    cosT = nc.declare_dram_parameter("cosT", [DH // 2, T], bf16, isOutput=False)
    sinT = nc.declare_dram_parameter("sinT", [DH // 2, T], bf16, isOutput=False)
    outp = nc.declare_dram_parameter("out", [BT, D], bf16, isOutput=True)

    HDH = DH + 1  # head slot width in v_ones (128 v cols + ones col)
    hh = DH // 2
    Copy = mybir.ActivationFunctionType.Copy

    with tile.TileContext(nc) as tc:
        with (
            tc.tile_pool(name="wpool", bufs=1) as wpool,
            tc.tile_pool(name="xpool", bufs=2) as xpool,
            tc.tile_pool(name="qkv", bufs=1) as qkv,
            tc.tile_pool(name="qkT", bufs=1) as qkT,
            tc.tile_pool(name="aop", bufs=2) as aop,
            tc.tile_pool(name="epool", bufs=2) as epool,
            tc.tile_pool(name="misc", bufs=2) as misc,
            tc.tile_pool(name="recp", bufs=4) as recp,
            tc.tile_pool(name="obuf", bufs=6) as obuf,
            tc.tile_pool(name="psS", bufs=2, space="PSUM") as psS,
            tc.tile_pool(name="ps512", bufs=2, space="PSUM") as ps512,
            tc.tile_pool(name="pspv", bufs=2, space="PSUM") as pspv,
        ):
            # ---- persistent: weights, tables, identity, bias columns ----
            ident = wpool.tile([P, P], bf16, tag="ident")
            make_identity(nc, ident)

            def load_wT(name, dram):
                # one DMA for all 16 k-chunks: [2048, DOUT] -> [128, 16, DOUT]
                wsb = wpool.tile([P, NK, DOUT], bf16, tag=name)
                wsrc = dram[:D, :].rearrange("(ko p) d -> p ko d", p=P)
                for c4 in range(4):
                    nc.gpsimd.dma_start(
                        out=wsb[:, c4 * 4 : (c4 + 1) * 4, :],
                        in_=wsrc[:, c4 * 4 : (c4 + 1) * 4, :],
                    )
                # bias as per-partition half-columns (base partition 0), per head
                bcols = []
                for m in range(HL):
                    halves = []
                    for h2 in range(2):
                        bc = wpool.tile([hh, 1], bf16, tag=f"{name}bc{m}_{h2}")
                        nc.gpsimd.dma_start(
                            out=bc,
                            in_=dram[D : D + 1, m * P + h2 * hh : m * P + (h2 + 1) * hh],
                        )
                        halves.append(bc)
                    bcols.append(halves)
                return wsb, bcols

            # k path loads first: attention's S matmuls need the FULL kT, so k
            # projections run before q each batch and their weights must land
            # first at startup. cos/sin go on the idle sync queue so RoPE
            # unblocks early without delaying the wk/x chunks.
            wk_t, wk_bc = load_wT("wk", wkT)
            cos_sb = wpool.tile([hh, T], bf16, tag="cos")
            sin_sb = wpool.tile([hh, T], bf16, tag="sin")
            deferred = {}

            def load_rest():
                wq_t, wq_bc = load_wT("wq", wqT)
                wv_t, _ = load_wT("wv", wvT)
                # v bias broadcast tile [P, DOUT] from the wvT bias row
                vb_bc = wpool.tile([P, DOUT], bf16, tag="vb_bc")
                wvT_brow = wvT[D : D + 1, :]
                nc.gpsimd.dma_start(
                    out=vb_bc,
                    in_=bass.AP(
                        tensor=wvT_brow.tensor,
                        offset=wvT_brow.offset,
                        ap=[[0, P], wvT_brow.ap[-1]],
                    ),
                )
                deferred.update(
                    wq_t=wq_t, wq_bc=wq_bc, wv_t=wv_t, vb_bc=vb_bc,
                )

            wo_t = []
            op_alt = [0]

            def emit_outproj_group(bb, tt, ds):
                t0 = bb * T + tt * P
                ps = ps512.tile([P, SPAN], f32, tag="p512", name="ps_op")
                for m in range(HL):
                    nc.tensor.matmul(
                        ps,
                        aoT_of[bb % 2][m][:, tt * P : (tt + 1) * P],
                        wo_t[m][:, ds * SPAN : (ds + 1) * SPAN],
                        start=(m == 0),
                        stop=(m == HL - 1),
                    )
                ob = obuf.tile([P, SPAN], bf16, tag="ob")
                op_alt[0] ^= 1
                if op_alt[0]:
                    nc.vector.tensor_copy(out=ob, in_=ps)
                    dma_eng = nc.sync
                else:
                    nc.scalar.copy(out=ob, in_=ps)
                    dma_eng = nc.gpsimd
                dma_eng.dma_start(
                    out=outp[t0 : t0 + P, ds * SPAN : (ds + 1) * SPAN], in_=ob
                )

            aoT_of = {}
            add = mybir.AluOpType.add
            mult = mybir.AluOpType.mult
            for b in range(B):
                # ---- QKV projections + RoPE, processed in span pairs ----
                qT = [qkT.tile([P, T], bf16, tag=f"qT{m}", name=f"qT{m}") for m in range(HL)]
                kT = [qkT.tile([P, T], bf16, tag=f"kT{m}", name=f"kT{m}") for m in range(HL)]
                v_t = [qkv.tile([P, HL * HDH], bf16, tag=f"v{tt}", name=f"v{tt}") for tt in range(NTT)]
                ao_t = [qkv.tile([P, DOUT], bf16, tag=f"ao{tt}", name=f"ao{tt}") for tt in range(NTT)]
                aoT_of[b % 2] = [aop.tile([P, T], bf16, tag=f"aoT{m}", name=f"aoT{m}") for m in range(HL)]

                xks = []
                for s2 in range(NSPAN // 2):
                    t0 = b * T + 2 * s2 * SPAN
                    xk = xpool.tile([P, NK, 2 * SPAN], bf16, tag="x", name="x")
                    xsrc = xT[:, t0 : t0 + 2 * SPAN].rearrange(
                        "(ko p) t -> p ko t", p=P
                    )
                    for c4 in range(4):
                        # at cold start split the first span-pair across both
                        # queues so the first k-projection unblocks sooner
                        eng = (
                            nc.sync
                            if (b == 0 and s2 == 0 and c4 >= 2)
                            else nc.gpsimd
                        )
                        eng.dma_start(
                            out=xk[:, c4 * 4 : (c4 + 1) * 4, :],
                            in_=xsrc[:, c4 * 4 : (c4 + 1) * 4, :],
                        )
                    xks.append(xk)
                    if not deferred:
                        # cos/sin ride the sync queue behind the first x
                        # chunks; RoPE doesn't need them until ~8us in
                        nc.sync.dma_start(out=cos_sb, in_=cosT[:, :])
                        nc.sync.dma_start(out=sin_sb, in_=sinT[:, :])
                        load_rest()
                if b == 0:
                    wq_t = deferred["wq_t"]
                    wq_bc = deferred["wq_bc"]
                    wv_t = deferred["wv_t"]
                    vb_bc = deferred["vb_bc"]

                # k/q over both span pairs (k first: S needs the full kT),
                # [128, 1024] psum per (dst, m, span-pair)
                for dst, wsb, bcols in ((kT, wk_t, wk_bc), (qT, wq_t, wq_bc)):
                    for s2 in range(NSPAN // 2):
                        xk = xks[s2]
                        sl2 = slice(2 * s2 * SPAN, (2 * s2 + 2) * SPAN)
                        for m in range(HL):
                            ps = psS.tile([P, 2 * SPAN], f32, tag="pS", name="ps_qk")
                            for k in range(NK):
                                for half in range(2):
                                    nc.tensor.matmul(
                                        ps[:, half * SPAN : (half + 1) * SPAN],
                                        wsb[:, k, m * P : (m + 1) * P],
                                        xk[:, k, half * SPAN : (half + 1) * SPAN],
                                        start=(k == 0),
                                        stop=(k == NK - 1),
                                    )
                            # RoPE over 1024 cols with bias fold:
                            #   qb = psum + bias; out[0:64] = qb0*cos - qb1*sin
                            #   out[64:128] = qb1*cos + qb0*sin
                            cs = cos_sb[:, sl2]
                            sn = sin_sb[:, sl2]
                            p0 = ps[0:hh, :]
                            p1 = ps[hh : 2 * hh, :]
                            b0 = bcols[m][0]
                            b1 = bcols[m][1]
                            t1 = misc.tile([hh, 2 * SPAN], bf16, tag="r1", name="t1")
                            t2 = misc.tile([hh, 2 * SPAN], bf16, tag="r2", name="t2")
                            nc.vector.scalar_tensor_tensor(t1, p1, b1, sn, add, mult)
                            nc.vector.scalar_tensor_tensor(t2, p0, b0, cs, add, mult)
                            nc.vector.tensor_sub(dst[m][0:hh, sl2], t2, t1)
                            t3 = misc.tile([hh, 2 * SPAN], bf16, tag="r1", name="t3")
                            t4 = misc.tile([hh, 2 * SPAN], bf16, tag="r2", name="t4")
                            nc.vector.scalar_tensor_tensor(t3, p0, b0, sn, add, mult)
                            nc.vector.scalar_tensor_tensor(t4, p1, b1, cs, add, mult)
                            nc.vector.tensor_add(dst[m][hh : 2 * hh, sl2], t4, t3)

                # v: per 128-token tile
                for s2 in range(NSPAN // 2):
                    xk = xks[s2]
                    for half in range(2):
                        s = 2 * s2 + half
                        for tt in range(SPAN // P):
                            gt = s * (SPAN // P) + tt
                            sl_p = slice(
                                half * SPAN + tt * P, half * SPAN + (tt + 1) * P
                            )
                            ps = ps512.tile([P, SPAN], f32, tag="p512", name="ps_v")
                            psv = ps[:, :DOUT]
                            for k in range(NK):
                                nc.tensor.matmul(
                                    psv,
                                    xk[:, k, sl_p],
                                    wv_t[:, k, :],
                                    start=(k == 0),
                                    stop=(k == NK - 1),
                                )
                            vt = v_t[gt]
                            ones_ap = bass.AP(
                                tensor=vt.tensor,
                                offset=vt.offset + DH,
                                ap=[vt.ap[0], [HDH, HL]],
                            )
                            nc.vector.memset(ones_ap, 1.0)
                            for m in range(HL):
                                nc.vector.tensor_add(
                                    vt[:, m * HDH : m * HDH + DH],
                                    psv[:, m * DH : (m + 1) * DH],
                                    vb_bc[:, m * DH : (m + 1) * DH],
                                )

                if b == 0:
                    # wo needed only from the first out-proj (during attn of b=1);
                    # late code position keeps it off the critical startup path.
                    for m in range(HL):
                        t = wpool.tile([P, D], bf16, tag=f"wo{m}")
                        nc.gpsimd.dma_start(out=t, in_=woT[m * P : (m + 1) * P, :])
                        wo_t.append(t)

                # ---- attention (b), span-outer, with out-proj interleaved ----
                # op_work: deferred out-proj groups of batch b-1, plus (for the
                # final batch) batch b's own groups appended as spans complete.
                op_work = (
                    [(b - 1, tt, ds) for tt in range(NTT) for ds in range(D // SPAN)]
                    if b > 0
                    else []
                )
                op_i = [0]
                op_rate = 2 if b == B - 1 else 1

                def emit_ops(n):
                    while op_i[0] < len(op_work) and n > 0:
                        emit_outproj_group(*op_work[op_i[0]])
                        op_i[0] += 1
                        n -= 1

                def pv_chain(m, s, etiles, tt):
                    gt = s * (SPAN // P) + tt
                    sl_p = slice(tt * P, (tt + 1) * P)
                    po = pspv.tile([P, DH + 1], f32, tag="pv", name="po")
                    for kt in range(NTT):
                        nc.tensor.matmul(
                            po,
                            etiles[kt][:, sl_p],
                            v_t[kt][:, m * HDH : (m + 1) * HDH],
                            start=(kt == 0),
                            stop=(kt == NTT - 1),
                        )
                    rec = recp.tile([P, 1], f32, tag="rec")
                    nc.vector.reciprocal(rec, po[:, DH : DH + 1])
                    nc.vector.tensor_scalar_mul(
                        ao_t[gt][:, m * DH : (m + 1) * DH],
                        po[:, 0:DH],
                        rec,
                    )

                def finish_span(s):
                    # transpose span s attn_out: [t, d] -> [d, t]; both heads'
                    # PV for span s have completed by the time this is called
                    for tt4 in range(SPAN // P):
                        tt = s * (SPAN // P) + tt4
                        for m in range(HL):
                            pt = pspv.tile([P, P], bf16, tag="pv", name="pt")
                            nc.tensor.transpose(
                                pt, ao_t[tt][:, m * DH : (m + 1) * DH], ident
                            )
                            nc.vector.tensor_copy(
                                out=aoT_of[b % 2][m][:, tt * P : (tt + 1) * P],
                                in_=pt,
                            )
                    if b == B - 1:
                        # final batch: its own out-proj drips right behind each
                        # span so the tail overlaps instead of serializing
                        for tt4 in range(SPAN // P):
                            tt = s * (SPAN // P) + tt4
                            for ds in range(D // SPAN):
                                op_work.append((b, tt, ds))

                # PV for half-span (m, s) is deferred into the NEXT half-span's
                # S/exp block (chains interleaved at odd kt2) so its exp inputs
                # are long since drained and the PE never starves on the psS
                # ring while the scalar engine works through the exps.
                pending_pv = None
                for s in range(NSPAN):
                    sl_q = slice(s * SPAN, (s + 1) * SPAN)
                    for m in range(HL):
                        # S.T pairs: two k-tiles per [P, 1024] psum -> one exp op
                        etiles = []
                        for kt2 in range(NTT // 2):
                            ps = psS.tile([P, 2 * SPAN], f32, tag="pS", name="ps_s")
                            for half in range(2):
                                nc.tensor.matmul(
                                    ps[:, half * SPAN : (half + 1) * SPAN],
                                    kT[m][:, (2 * kt2 + half) * P : (2 * kt2 + half + 1) * P],
                                    qT[m][:, sl_q],
                                    start=True,
                                    stop=True,
                                )
                            e = epool.tile([P, 2 * SPAN], bf16, tag=f"e{kt2}")
                            nc.scalar.activation(
                                out=e, in_=ps, func=mybir.ActivationFunctionType.Exp
                            )
                            etiles.append(e[:, 0:SPAN])
                            etiles.append(e[:, SPAN : 2 * SPAN])
                            # keep PE fed while exp drains the S psum slots
                            emit_ops(op_rate)
                            if kt2 % 2 == 1 and pending_pv is not None:
                                pv_chain(*pending_pv, kt2 // 2)
                        if pending_pv is not None and pending_pv[0] == HL - 1:
                            finish_span(pending_pv[1])
                        pending_pv = (m, s, etiles)
                # batch tail: final half-span's chains, transposes, and (for
                # the last batch) out-proj groups interleaved per token-tile
                # so PE work overlaps the trailing DVE/DMA drains.
                s_l = NSPAN - 1
                for tt4 in range(SPAN // P):
                    pv_chain(*pending_pv, tt4)
                    tt = s_l * (SPAN // P) + tt4
                    for m in range(HL):
                        pt = pspv.tile([P, P], bf16, tag="pv", name="pt")
                        nc.tensor.transpose(
                            pt, ao_t[tt][:, m * DH : (m + 1) * DH], ident
                        )
                        nc.vector.tensor_copy(
                            out=aoT_of[b % 2][m][:, tt * P : (tt + 1) * P],
                            in_=pt,
                        )
                    if b == B - 1:
                        for ds in range(D // SPAN):
                            op_work.append((b, tt, ds))
                        emit_ops(D // SPAN)
                emit_ops(len(op_work))
    return nc


_nc_cache = None


def _get_nc():
    global _nc_cache
    if _nc_cache is None:
        _nc_cache = _build_nc()
    return _nc_cache


# ---------------------------------------------------------------------------
# Host wrapper
# ---------------------------------------------------------------------------


def _prep_inputs(x, pos, wq_w, wq_b, wk_w, wk_b, wv_w, wv_b, wo_w, wo_b):
    x2 = np.asarray(x, np.float32).reshape(BT, D)
    xT = np.ascontiguousarray(x2.T).astype(BF16)

    pos1 = np.asarray(pos, np.float32).reshape(T)
    freq = (1.0 / 10000.0 ** (np.arange(0, DH, 2, np.float32) / DH)).astype(np.float32)
    ang = pos1[None, :] * freq[:, None]  # [64, T]
    cosT = np.cos(ang).astype(BF16)
    sinT = np.sin(ang).astype(BF16)

    scale = np.float32(1.0 / np.sqrt(DH))

    def wslice(w, bvec, c, s=None):
        w = np.asarray(w, np.float32)
        bvec = np.asarray(bvec, np.float32)
        ws = w[c * DOUT : (c + 1) * DOUT]  # [256, D]
        bs = bvec[c * DOUT : (c + 1) * DOUT]
        if s is not None:
            ws = ws * s
            bs = bs * s
        out = np.empty((D + 1, DOUT), BF16)
        out[:D] = ws.T.astype(BF16)
        out[D] = bs.astype(BF16)
        return out

    in_maps = []
    for c in range(N_CORES):
        woTc = (
            np.asarray(wo_w, np.float32)[:, c * DOUT : (c + 1) * DOUT]
            .T.astype(BF16)
            .copy()
        )
        in_maps.append(
            {
                "xT": xT,
                "wqT": wslice(wq_w, wq_b, c, scale),
                "wkT": wslice(wk_w, wk_b, c),
                "wvT": wslice(wv_w, wv_b, c),
                "woT": woTc,
                "cosT": cosT,
                "sinT": sinT,
            }
        )
    return in_maps


def _run(in_maps, trace=False):
    _install_compile_patch()
    from concourse.bass_utils import run_bass_kernel_spmd

    nc = _get_nc()
    return run_bass_kernel_spmd(
        nc, in_maps, core_ids=list(range(N_CORES)), trace=trace
    )


def kernel(**inputs):
    inputs = {k: np.asarray(v) for k, v in inputs.items()}
    in_maps = _prep_inputs(**inputs)
    r = _run(in_maps, trace=False)
    acc = np.zeros((BT, D), np.float32)
    for c in range(N_CORES):
        acc += r.results[c]["out"].astype(np.float32)
    acc += np.asarray(inputs["wo_b"], np.float32)
    return acc.reshape(B, T, D)


# revision 33
# speedup vs baseline: 1.1208x; 1.0041x over previous
"""Multi-head attention (B=4, T=2048, dim=2048, H=16, RoPE) on 8 TRN2 NeuronCores.

Tensor-parallel over heads: core c owns heads {2c, 2c+1} (projection dim
slice [256c, 256c+256)).  Each core computes q/k/v projections for its
heads, RoPE, full softmax attention for its 8 (batch, head) pairs, and a
partial output projection against its 256-row slice of wo; the host sums
the 8 partial outputs (bf16) and adds wo_b.

All matmuls run in bf16 with f32 PSUM accumulation; softmax runs exp in
f32->bf16 on the scalar engine with denominators accumulated via an extra
ones-column on V through the PV matmul.

Engine assignment: PE does all matmuls/transposes; Activation does exp
(the critical softmax chain) plus half the out-proj PSUM drains; DVE does
RoPE, reciprocals, PV scale, aoT copies, and the other out-proj drains;
gpsimd issues the weight/x DMAs and half the output DMAs (it cannot touch
PSUM on TRN2).  The attention loop is span-outer and PV for each
half-span is deferred into the next half-span's S/exp block (chains
interleaved at odd kt2) so the PE never stalls on the exp pipeline;
deferred out-proj groups of the previous batch fill the remaining slots,
and the final batch's own out-proj drips per token-tile at the tail.
"""

import json
import sys

sys.path.insert(0, "/opt/trn_rl_repo")

import ml_dtypes
import numpy as np

BF16 = ml_dtypes.bfloat16

# Problem shape (hardcoded per contract).
B, T, D = 4, 2048, 2048
H = 16
N_CORES = 8
HL = H // N_CORES  # heads per core = 2
DH = D // H  # head dim = 128
DOUT = HL * DH  # per-core projection width = 256
BT = B * T  # 8192 tokens
P = 128
NK = D // P  # 16 feature chunks
SPAN = 512
NSPAN = T // SPAN  # 4 token spans per batch
NTT = T // P  # 16 token tiles per batch


# ---------------------------------------------------------------------------
# BIR legalization: the walrus build in this container rejects instructions
# carrying more than one sync wait. Engines execute their stream in order, so
# hoisting excess waits into standalone EventSemaphore instructions directly
# before the instruction (same engine) is semantically equivalent; Tile's
# dependency graph is acyclic so this cannot deadlock.
# ---------------------------------------------------------------------------


def _legalize_waits(bir_json: bytes, max_inline: int = 1, es_capacity: int = 2):
    bir = json.loads(bir_json)
    for f in bir.get("functions", []):
        for bb in f.get("blocks", []):
            out = []
            for inst in bb.get("instructions", []):
                si = inst.get("sync_info")
                waits = (si or {}).get("on_wait") or []
                cap = (
                    es_capacity
                    if inst.get("opcode") == "EventSemaphore"
                    else max_inline
                )
                if len(waits) > cap:
                    keep, excess = waits[:cap], waits[cap:]
                    for ci in range(0, len(excess), es_capacity):
                        out.append(
                            {
                                "debug": inst.get("debug", 0),
                                "engine": inst["engine"],
                                "ins": [],
                                "name": f"{inst['name']}_xw{ci}",
                                "opcode": "EventSemaphore",
                                "outs": [],
                                "sync_info": {
                                    "on_update": [],
                                    "on_wait": excess[ci : ci + es_capacity],
                                },
                            }
                        )
                    si["on_wait"] = keep
                out.append(inst)
            bb["instructions"] = out
    return json.dumps(bir).encode()


_patched = False


def _install_compile_patch():
    global _patched
    if _patched:
        return
    _patched = True
    from concourse import bass2jax, bass_utils

    orig = bass_utils.compile_bir_kernel

    def patched_compile(bir_json, tmpdir, neff_name="file.neff"):
        return orig(_legalize_waits(bir_json), tmpdir, neff_name)

    bass2jax.compile_bir_kernel = patched_compile


# ---------------------------------------------------------------------------
# Kernel builder (one SPMD graph; per-core behavior differs only via inputs)
# ---------------------------------------------------------------------------


def _build_nc():
    import concourse.bass as bass
    import concourse.tile as tile
    from concourse import mybir
    from concourse.masks import make_identity

    f32 = mybir.dt.float32
    bf16 = mybir.dt.bfloat16

    nc = bass.Bass()
    xT = nc.declare_dram_parameter("xT", [D, BT], bf16, isOutput=False)
    wqT = nc.declare_dram_parameter("wqT", [D + 1, DOUT], bf16, isOutput=False)
    wkT = nc.declare_dram_parameter("wkT", [D + 1, DOUT], bf16, isOutput=False)
    wvT = nc.declare_dram_parameter("wvT", [D + 1, DOUT], bf16, isOutput=False)
    woT = nc.declare_dram_parameter("woT", [DOUT, D], bf16, isOutput=False)
    cosT = nc.declare_dram_parameter("cosT", [DH // 2, T], bf16, isOutput=False)
    sinT = nc.declare_dram_parameter("sinT", [DH // 2, T], bf16, isOutput=False)
    outp = nc.declare_dram_parameter("out", [BT, D], bf16, isOutput=True)

    HDH = DH + 1  # head slot width in v_ones (128 v cols + ones col)
    hh = DH // 2
    Copy = mybir.ActivationFunctionType.Copy

    with tile.TileContext(nc) as tc:
        with (
            tc.tile_pool(name="wpool", bufs=1) as wpool,
            tc.tile_pool(name="xpool", bufs=2) as xpool,
            tc.tile_pool(name="qkv", bufs=1) as qkv,
            tc.tile_pool(name="qkT", bufs=1) as qkT,
            tc.tile_pool(name="aop", bufs=2) as aop,
            tc.tile_pool(name="epool", bufs=2) as epool,
            tc.tile_pool(name="misc", bufs=2) as misc,
            tc.tile_pool(name="recp", bufs=4) as recp,
            tc.tile_pool(name="obuf", bufs=6) as obuf,
            tc.tile_pool(name="psS", bufs=2, space="PSUM") as psS,
            tc.tile_pool(name="ps512", bufs=2, space="PSUM") as ps512,
            tc.tile_pool(name="pspv", bufs=2, space="PSUM") as pspv,
        ):
            # ---- persistent: weights, tables, identity, bias columns ----
            ident = wpool.tile([P, P], bf16, tag="ident")
            make_identity(nc, ident)

            def load_wT(name, dram):
                # one DMA for all 16 k-chunks: [2048, DOUT] -> [128, 16, DOUT]
                wsb = wpool.tile([P, NK, DOUT], bf16, tag=name)
                wsrc = dram[:D, :].rearrange("(ko p) d -> p ko d", p=P)
                for c4 in range(4):
                    nc.gpsimd.dma_start(
                        out=wsb[:, c4 * 4 : (c4 + 1) * 4, :],
                        in_=wsrc[:, c4 * 4 : (c4 + 1) * 4, :],
                    )
                # bias as per-partition half-columns (base partition 0), per head
                bcols = []
                for m in range(HL):
                    halves = []
                    for h2 in range(2):
                        bc = wpool.tile([hh, 1], bf16, tag=f"{name}bc{m}_{h2}")
                        nc.gpsimd.dma_start(
                            out=bc,
                            in_=dram[D : D + 1, m * P + h2 * hh : m * P + (h2 + 1) * hh],
                        )
                        halves.append(bc)
                    bcols.append(halves)
                return wsb, bcols

            # k path loads first: attention's S matmuls need the FULL kT, so k
            # projections run before q each batch and their weights must land
            # first at startup. cos/sin go on the idle sync queue so RoPE
            # unblocks early without delaying the wk/x chunks.
            wk_t, wk_bc = load_wT("wk", wkT)
            cos_sb = wpool.tile([hh, T], bf16, tag="cos")
            sin_sb = wpool.tile([hh, T], bf16, tag="sin")
            deferred = {}

            def load_rest():
                wq_t, wq_bc = load_wT("wq", wqT)
                wv_t, _ = load_wT("wv", wvT)
                # v bias broadcast tile [P, DOUT] from the wvT bias row
                vb_bc = wpool.tile([P, DOUT], bf16, tag="vb_bc")
                wvT_brow = wvT[D : D + 1, :]
                nc.gpsimd.dma_start(
                    out=vb_bc,
                    in_=bass.AP(
                        tensor=wvT_brow.tensor,
                        offset=wvT_brow.offset,
                        ap=[[0, P], wvT_brow.ap[-1]],
                    ),
                )
                deferred.update(
                    wq_t=wq_t, wq_bc=wq_bc, wv_t=wv_t, vb_bc=vb_bc,
                )

            wo_t = []
            op_alt = [0]

            def emit_outproj_group(bb, tt, ds):
                t0 = bb * T + tt * P
                ps = ps512.tile([P, SPAN], f32, tag="p512", name="ps_op")
                for m in range(HL):
                    nc.tensor.matmul(
                        ps,
                        aoT_of[bb % 2][m][:, tt * P : (tt + 1) * P],
                        wo_t[m][:, ds * SPAN : (ds + 1) * SPAN],
                        start=(m == 0),
                        stop=(m == HL - 1),
                    )
                ob = obuf.tile([P, SPAN], bf16, tag="ob")
                op_alt[0] ^= 1
                if op_alt[0]:
                    nc.vector.tensor_copy(out=ob, in_=ps)
                    dma_eng = nc.sync
                else:
                    nc.scalar.copy(out=ob, in_=ps)
                    dma_eng = nc.gpsimd
                dma_eng.dma_start(
                    out=outp[t0 : t0 + P, ds * SPAN : (ds + 1) * SPAN], in_=ob
                )

            aoT_of = {}
            add = mybir.AluOpType.add
            mult = mybir.AluOpType.mult
            for b in range(B):
                # ---- QKV projections + RoPE, processed in span pairs ----
                qT = [qkT.tile([P, T], bf16, tag=f"qT{m}", name=f"qT{m}") for m in range(HL)]
                kT = [qkT.tile([P, T], bf16, tag=f"kT{m}", name=f"kT{m}") for m in range(HL)]
                v_t = [qkv.tile([P, HL * HDH], bf16, tag=f"v{tt}", name=f"v{tt}") for tt in range(NTT)]
                ao_t = [qkv.tile([P, DOUT], bf16, tag=f"ao{tt}", name=f"ao{tt}") for tt in range(NTT)]
                aoT_of[b % 2] = [aop.tile([P, T], bf16, tag=f"aoT{m}", name=f"aoT{m}") for m in range(HL)]

                xks = []
                for s2 in range(NSPAN // 2):
                    t0 = b * T + 2 * s2 * SPAN
                    xk = xpool.tile([P, NK, 2 * SPAN], bf16, tag="x", name="x")
                    xsrc = xT[:, t0 : t0 + 2 * SPAN].rearrange(
                        "(ko p) t -> p ko t", p=P
                    )
                    for c4 in range(4):
                        # at cold start split the first span-pair across both
                        # queues so the first k-projection unblocks sooner
                        eng = (
                            nc.sync
                            if (b == 0 and c4 >= 2)
                            else nc.gpsimd
                        )
                        eng.dma_start(
                            out=xk[:, c4 * 4 : (c4 + 1) * 4, :],
                            in_=xsrc[:, c4 * 4 : (c4 + 1) * 4, :],
                        )
                    xks.append(xk)
                    if not deferred:
                        # cos/sin ride the sync queue behind the first x
                        # chunks; RoPE doesn't need them until ~8us in
                        nc.sync.dma_start(out=cos_sb, in_=cosT[:, :])
                        nc.sync.dma_start(out=sin_sb, in_=sinT[:, :])
                        load_rest()
                if b == 0:
                    wq_t = deferred["wq_t"]
                    wq_bc = deferred["wq_bc"]
                    wv_t = deferred["wv_t"]
                    vb_bc = deferred["vb_bc"]

                # k/q over both span pairs (k first: S needs the full kT),
                # [128, 1024] psum per (dst, m, span-pair)
                for dst, wsb, bcols in ((kT, wk_t, wk_bc), (qT, wq_t, wq_bc)):
                    for s2 in range(NSPAN // 2):
                        xk = xks[s2]
                        sl2 = slice(2 * s2 * SPAN, (2 * s2 + 2) * SPAN)
                        for m in range(HL):
                            ps = psS.tile([P, 2 * SPAN], f32, tag="pS", name="ps_qk")
                            for k in range(NK):
                                for half in range(2):
                                    nc.tensor.matmul(
                                        ps[:, half * SPAN : (half + 1) * SPAN],
                                        wsb[:, k, m * P : (m + 1) * P],
                                        xk[:, k, half * SPAN : (half + 1) * SPAN],
                                        start=(k == 0),
                                        stop=(k == NK - 1),
                                    )
                            # RoPE over 1024 cols with bias fold:
                            #   qb = psum + bias; out[0:64] = qb0*cos - qb1*sin
                            #   out[64:128] = qb1*cos + qb0*sin
                            cs = cos_sb[:, sl2]
                            sn = sin_sb[:, sl2]
                            p0 = ps[0:hh, :]
                            p1 = ps[hh : 2 * hh, :]
                            b0 = bcols[m][0]
                            b1 = bcols[m][1]
                            t1 = misc.tile([hh, 2 * SPAN], bf16, tag="r1", name="t1")
                            t2 = misc.tile([hh, 2 * SPAN], bf16, tag="r2", name="t2")
                            nc.vector.scalar_tensor_tensor(t1, p1, b1, sn, add, mult)
                            nc.vector.scalar_tensor_tensor(t2, p0, b0, cs, add, mult)
                            nc.vector.tensor_sub(dst[m][0:hh, sl2], t2, t1)
                            t3 = misc.tile([hh, 2 * SPAN], bf16, tag="r1", name="t3")
                            t4 = misc.tile([hh, 2 * SPAN], bf16, tag="r2", name="t4")
                            nc.vector.scalar_tensor_tensor(t3, p0, b0, sn, add, mult)
                            nc.vector.scalar_tensor_tensor(t4, p1, b1, cs, add, mult)
                            nc.vector.tensor_add(dst[m][hh : 2 * hh, sl2], t4, t3)

                # v: per 128-token tile
                for s2 in range(NSPAN // 2):
                    xk = xks[s2]
                    for half in range(2):
                        s = 2 * s2 + half
                        for tt in range(SPAN // P):
                            gt = s * (SPAN // P) + tt
                            sl_p = slice(
                                half * SPAN + tt * P, half * SPAN + (tt + 1) * P
                            )
                            ps = ps512.tile([P, SPAN], f32, tag="p512", name="ps_v")
                            psv = ps[:, :DOUT]
                            for k in range(NK):
                                nc.tensor.matmul(
                                    psv,
                                    xk[:, k, sl_p],
                                    wv_t[:, k, :],
                                    start=(k == 0),
                                    stop=(k == NK - 1),
                                )
                            vt = v_t[gt]
                            ones_ap = bass.AP(
                                tensor=vt.tensor,
                                offset=vt.offset + DH,
                                ap=[vt.ap[0], [HDH, HL]],
                            )
                            nc.vector.memset(ones_ap, 1.0)
                            for m in range(HL):
                                nc.vector.tensor_add(
                                    vt[:, m * HDH : m * HDH + DH],
                                    psv[:, m * DH : (m + 1) * DH],
                                    vb_bc[:, m * DH : (m + 1) * DH],
                                )

                if b == 0:
                    # wo needed only from the first out-proj (during attn of b=1);
                    # late code position keeps it off the critical startup path.
                    for m in range(HL):
                        t = wpool.tile([P, D], bf16, tag=f"wo{m}")
                        nc.gpsimd.dma_start(out=t, in_=woT[m * P : (m + 1) * P, :])
                        wo_t.append(t)

                # ---- attention (b), span-outer, with out-proj interleaved ----
                # op_work: deferred out-proj groups of batch b-1, plus (for the
                # final batch) batch b's own groups appended as spans complete.
                op_work = (
                    [(b - 1, tt, ds) for tt in range(NTT) for ds in range(D // SPAN)]
                    if b > 0
                    else []
                )
                op_i = [0]
                op_rate = 2 if b == B - 1 else 1

                def emit_ops(n):
                    while op_i[0] < len(op_work) and n > 0:
                        emit_outproj_group(*op_work[op_i[0]])
                        op_i[0] += 1
                        n -= 1

                def pv_chain(m, s, etiles, tt):
                    gt = s * (SPAN // P) + tt
                    sl_p = slice(tt * P, (tt + 1) * P)
                    po = pspv.tile([P, DH + 1], f32, tag="pv", name="po")
                    for kt in range(NTT):
                        nc.tensor.matmul(
                            po,
                            etiles[kt][:, sl_p],
                            v_t[kt][:, m * HDH : (m + 1) * HDH],
                            start=(kt == 0),
                            stop=(kt == NTT - 1),
                        )
                    rec = recp.tile([P, 1], f32, tag="rec")
                    nc.vector.reciprocal(rec, po[:, DH : DH + 1])
                    nc.vector.tensor_scalar_mul(
                        ao_t[gt][:, m * DH : (m + 1) * DH],
                        po[:, 0:DH],
                        rec,
                    )

                fin_q = []

                def emit_transpose(tt, m):
                    pt = pspv.tile([P, P], bf16, tag="pv", name="pt")
                    nc.tensor.transpose(
                        pt, ao_t[tt][:, m * DH : (m + 1) * DH], ident
                    )
                    nc.vector.tensor_copy(
                        out=aoT_of[b % 2][m][:, tt * P : (tt + 1) * P],
                        in_=pt,
                    )
                    if m == HL - 1 and b == B - 1:
                        # final batch: this tile's out-proj becomes available
                        # only once BOTH its aoT transposes are in the stream
                        for ds in range(D // SPAN):
                            op_work.append((b, tt, ds))

                def finish_span(s):
                    # queue span s transposes ([t, d] -> [d, t]) to drain one
                    # per kt2 slot, spreading the PE/DVE burst; both heads'
                    # PV for span s have completed by the time they drain
                    for tt4 in range(SPAN // P):
                        tt = s * (SPAN // P) + tt4
                        for m in range(HL):
                            fin_q.append((tt, m))

                # PV for half-span (m, s) is deferred into the NEXT half-span's
                # S/exp block (chains interleaved at odd kt2) so its exp inputs
                # are long since drained and the PE never starves on the psS
                # ring while the scalar engine works through the exps.
                pending_pv = None
                for s in range(NSPAN):
                    sl_q = slice(s * SPAN, (s + 1) * SPAN)
                    for m in range(HL):
                        # S.T pairs: two k-tiles per [P, 1024] psum -> one exp op
                        etiles = []
                        for kt2 in range(NTT // 2):
                            ps = psS.tile([P, 2 * SPAN], f32, tag="pS", name="ps_s")
                            for half in range(2):
                                nc.tensor.matmul(
                                    ps[:, half * SPAN : (half + 1) * SPAN],
                                    kT[m][:, (2 * kt2 + half) * P : (2 * kt2 + half + 1) * P],
                                    qT[m][:, sl_q],
                                    start=True,
                                    stop=True,
                                )
                            e = epool.tile([P, 2 * SPAN], bf16, tag=f"e{kt2}")
                            nc.scalar.activation(
                                out=e, in_=ps, func=mybir.ActivationFunctionType.Exp
                            )
                            etiles.append(e[:, 0:SPAN])
                            etiles.append(e[:, SPAN : 2 * SPAN])
                            # keep PE fed while exp drains the S psum slots
                            emit_ops(op_rate)
                            if fin_q:
                                emit_transpose(*fin_q.pop(0))
                            if kt2 % 2 == 1 and pending_pv is not None:
                                pv_chain(*pending_pv, kt2 // 2)
                        if pending_pv is not None and pending_pv[0] == HL - 1:
                            finish_span(pending_pv[1])
                        pending_pv = (m, s, etiles)
                # batch tail: final half-span's chains, transposes, and (for
                # the last batch) out-proj groups interleaved per token-tile
                # so PE work overlaps the trailing DVE/DMA drains.
                s_l = NSPAN - 1
                while fin_q:
                    emit_transpose(*fin_q.pop(0))
                for tt4 in range(SPAN // P):
                    pv_chain(*pending_pv, tt4)
                    tt = s_l * (SPAN // P) + tt4
                    for m in range(HL):
                        emit_transpose(tt, m)
                    emit_ops(D // SPAN)
                emit_ops(len(op_work))
    return nc


_nc_cache = None


def _get_nc():
    global _nc_cache
    if _nc_cache is None:
        _nc_cache = _build_nc()
    return _nc_cache


# ---------------------------------------------------------------------------
# Host wrapper
# ---------------------------------------------------------------------------


def _prep_inputs(x, pos, wq_w, wq_b, wk_w, wk_b, wv_w, wv_b, wo_w, wo_b):
    x2 = np.asarray(x, np.float32).reshape(BT, D)
    xT = np.ascontiguousarray(x2.T).astype(BF16)

    pos1 = np.asarray(pos, np.float32).reshape(T)
    freq = (1.0 / 10000.0 ** (np.arange(0, DH, 2, np.float32) / DH)).astype(np.float32)
    ang = pos1[None, :] * freq[:, None]  # [64, T]
    cosT = np.cos(ang).astype(BF16)
    sinT = np.sin(ang).astype(BF16)

    scale = np.float32(1.0 / np.sqrt(DH))

    def wslice(w, bvec, c, s=None):
        w = np.asarray(w, np.float32)
        bvec = np.asarray(bvec, np.float32)
        ws = w[c * DOUT : (c + 1) * DOUT]  # [256, D]
        bs = bvec[c * DOUT : (c + 1) * DOUT]
        if s is not None:
            ws = ws * s
            bs = bs * s
        out = np.empty((D + 1, DOUT), BF16)
        out[:D] = ws.T.astype(BF16)
        out[D] = bs.astype(BF16)
        return out

    in_maps = []
    for c in range(N_CORES):
        woTc = (
            np.asarray(wo_w, np.float32)[:, c * DOUT : (c + 1) * DOUT]
            .T.astype(BF16)
            .copy()
        )
        in_maps.append(
            {
                "xT": xT,
                "wqT": wslice(wq_w, wq_b, c, scale),
                "wkT": wslice(wk_w, wk_b, c),
                "wvT": wslice(wv_w, wv_b, c),
                "woT": woTc,
                "cosT": cosT,
                "sinT": sinT,
            }
        )
    return in_maps


def _run(in_maps, trace=False):
    _install_compile_patch()
    from concourse.bass_utils import run_bass_kernel_spmd

    nc = _get_nc()
    return run_bass_kernel_spmd(
        nc, in_maps, core_ids=list(range(N_CORES)), trace=trace
    )


def kernel(**inputs):
    inputs = {k: np.asarray(v) for k, v in inputs.items()}
    in_maps = _prep_inputs(**inputs)
    r = _run(in_maps, trace=False)
    acc = np.zeros((BT, D), np.float32)
    for c in range(N_CORES):
        acc += r.results[c]["out"].astype(np.float32)
    acc += np.asarray(inputs["wo_b"], np.float32)
    return acc.reshape(B, T, D)
